# revision 35
# baseline (speedup 1.0000x reference)
"""Trainium2 Bass kernel for nn_Classifier_22625887715977 (sparse_attention).

kernel(**inputs) takes FULL unsharded inputs (bs=32) and returns the full
[32, 75, 6] logits. Shards the batch over 8 NeuronCores (4 episodes per core).

Math (per episode, exact reassociation of the reference):
  s      = leaky(ss @ Wm1 + bm1) @ Wm2 + bm2
  avg    = mean_n [bw | bsm]
  gvis   = sigmoid(avg @ Wvis + bvis) + 1 ; gsem = sigmoid(avg @ Wsem + bsem) + 1
  q      = sc @ Wq + s @ Wqs
  scores = ((q @ Wk^T) * gvis) @ bw^T + ((q @ Wks^T) * gsem) @ bsm^T ; attn = softmax(scores/32)
  out    = ((attn @ bw) * gvis) @ Wv ; out2 = out @ Wfc + sc
  fake   = mean_w out2 ; protos = [sc; fake] ; logits = temp * cos(qf, protos)

v12 implementation notes:
 - Mixed precision chosen by an error-budget bisect: fp8-e4m3 only where the
   output is insensitive (gate weights Wvis/Wsem + avgT, and the value-path
   weights Wv/Wfc with ugT/outT); everything feeding scores stays bf16; the
   residual / fake-proto path stays f32. Predicted rel-err ~1.2e-2.
 - All heavy projections are computed DIRECTLY in transposed layout (tiny
   matmul free sizes): qT, t1T, t2T, avgT (bank mean via matmul against a
   1/512 selector block), gate pre-activations, uT, outT.
 - Banks load once (bf16 natural) and are PE-transposed on-chip during the
   DMA window; copies split across DVE/Act.
 - Sigmoid via Exp (+DVE recip) so Act needs only {Sqrt, Exp} tables; dummy
   ops prime the table switches off the critical path.
 - Single sync-queue DMA stream interleaved in need-order; qf arrives
   host-transposed and packed, its norm folded into a final per-query scale.
 - Per-episode attention runs stage-parallel to hide cross-engine latency.
"""

import numpy as np
import ml_dtypes

BF16 = ml_dtypes.bfloat16
FP8 = ml_dtypes.float8_e4m3

BS = 32
NCORES = 8
EPC = BS // NCORES          # 4
NW = 5
B20 = EPC * NW              # 20
FD = 1024
FDC = FD // 128             # 8
SEM = 300
SEMCH = [(0, 128), (128, 128), (256, 44)]
SEMP = 384
NB = 512
NBC = NB // 128             # 4
NQ = 75
NPROTO = NW + 1             # 6
VINC = 11
VINP = VINC * 128           # 1408

_MODULE_CACHE = {}


def _build_module(temp: float):
    import concourse.mybir as mybir
    import concourse.tile as tile
    from concourse import bacc

    f32 = mybir.dt.float32
    f32r = mybir.dt.float32r
    bf = mybir.dt.bfloat16
    f8 = mybir.dt.float8e4
    AF = mybir.ActivationFunctionType
    ALU = mybir.AluOpType

    nc = bacc.Bacc("TRN2", target_bir_lowering=False, debug=False)

    def di(name, shape, dt=f32):
        return nc.dram_tensor(name, shape, dt, kind="ExternalInput")

    auxbf_d = di("aux_bf", [128, 2069], bf)
    auxf8_d = di("aux_f8", [128, 1504], f8)
    auxf32_d = di("aux_f32", [128, 1035])
    fifths_d = di("fifths", [B20, EPC])
    wq_d = di("Wq", [FD, FD], bf)
    wqs_d = di("Wqs_pad", [SEMP, FD], bf)
    wkT_d = di("WkT", [FD, FD], bf)
    wksT_d = di("WksT_pack", [128, FDC * SEM], bf)
    wv_d = di("Wv", [FD, FD], f8)
    wfc_d = di("Wfc", [FD, FD], f8)
    wvis_d = di("Wvis_pad", [VINP, FD], f8)
    wsem_d = di("Wsem_pack", [128, VINC * SEM], f8)
    bw_d = di("bw", [EPC, NB, FD], bf)
    bsm_d = di("bsm_pack", [EPC, 128, NBC * SEM], bf)
    qfT_d = di("qfT_pack", [EPC, 128, FDC * NQ], bf)
    out_d = nc.dram_tensor("out", [EPC, NQ, NPROTO], f32, kind="ExternalOutput")

    from contextlib import ExitStack
    with tile.TileContext(nc) as tc, ExitStack() as _ctx:
        def _pool(**kw):
            return _ctx.enter_context(tc.tile_pool(**kw))

        cpool = _pool(name="const", bufs=1)
        wres = _pool(name="wres", bufs=1)
        wbig = _pool(name="wbig", bufs=2)
        wvp = _pool(name="wvis", bufs=1)
        bpool = _pool(name="banks", bufs=1)
        tpool = _pool(name="bankT", bufs=1)
        apool = _pool(name="acts", bufs=1)
        spool = _pool(name="small", bufs=1)
        ep4 = _pool(name="ep4", bufs=4)
        qpool = _pool(name="qn", bufs=1)
        pbt = _pool(name="pbt", bufs=2, space="PSUM")
        pmisc = _pool(name="pmisc", bufs=2, space="PSUM")
        pavt = _pool(name="pavt", bufs=1, space="PSUM")
        pacc = _pool(name="pacc", bufs=1, space="PSUM")
        pscore = _pool(name="pscore", bufs=2, space="PSUM")

        # ---------------- DMA issue ----------------
        # gpsimd (SWDGE): just fifths (tiny); Pool otherwise idle
        fifths = cpool.tile([B20, EPC], f32r, tag="fifths")
        nc.gpsimd.dma_start(fifths[:], fifths_d.ap().bitcast(f32r))

        # sync (SP): single stream, interleaved in need-order
        aux_bf = cpool.tile([128, 2069], bf, tag="aux_bf")
        nc.sync.dma_start(aux_bf[:], auxbf_d.ap())
        aux_f8 = cpool.tile([128, 1504], f8, tag="aux_f8")
        nc.sync.dma_start(aux_f8[:], auxf8_d.ap())
        aux_f32 = cpool.tile([128, 1035], f32, tag="aux_f32")
        nc.sync.dma_start(aux_f32[:], auxf32_d.ap())
        ident = aux_bf[:, 0:128]
        ssT = aux_bf[:, 128:188].rearrange("p (c b) -> p c b", c=3)
        esel = aux_bf[:, 188:268].rearrange("p (e b) -> p e b", e=EPC)
        wm1 = aux_bf[:, 268:1168].rearrange("p (c d) -> p c d", c=3)
        wm2 = aux_bf[:, 1168:2068].rearrange("p (c d) -> p c d", c=3)
        ones128 = aux_bf[:, 2068:2069]
        ones20 = aux_f8[0:1, 160:180]
        bvis_row = aux_f8[0:1, 180:1204]
        bsem_row = aux_f8[0:1, 1204:1504]
        sc_nat = aux_f32[0:B20, 0:FD]
        bm1c = aux_f32[:, 1024:1027]
        bm2c = aux_f32[:, 1027:1030]

        bw_nat, bsm_nat = [None] * EPC, [None] * EPC
        wq = wbig.tile([128, FDC, FD], bf, tag="wbig")
        wkT = wbig.tile([128, FDC, FD], bf, tag="wbig")
        wqs = wres.tile([128, 3, FD], bf, tag="wqs")
        wksT = wres.tile([128, FDC, SEM], bf, tag="wksT")
        wvis = wvp.tile([128, VINC, FD], f8, tag="wvis")
        wsem = wvp.tile([128, VINC, SEM], f8, tag="wsem")

        def load_bank(e):
            bwt = bpool.tile([128, NBC, FD], bf, tag=f"bw{e}")
            nc.sync.dma_start(bwt[:], bw_d.ap()[e].rearrange("(c p) d -> p c d", p=128))
            bw_nat[e] = bwt
            bst = bpool.tile([128, NBC, SEM], bf, tag=f"bsm{e}")
            nc.sync.dma_start(bst[:], bsm_d.ap()[e].rearrange("p (c d) -> p c d", c=NBC))
            bsm_nat[e] = bst

        load_bank(0)
        load_bank(1)
        nc.sync.dma_start(wq[:], wq_d.ap().rearrange("(c p) d -> p c d", p=128))
        load_bank(2)
        nc.sync.dma_start(wqs[:], wqs_d.ap().rearrange("(c p) d -> p c d", p=128))
        nc.sync.dma_start(wkT[:], wkT_d.ap().rearrange("(c p) d -> p c d", p=128))
        load_bank(3)
        nc.sync.dma_start(wksT[:], wksT_d.ap().rearrange("p (c d) -> p c d", c=FDC))
        nc.sync.dma_start(wvis[:], wvis_d.ap().rearrange("(c p) d -> p c d", p=128))
        nc.sync.dma_start(wsem[:], wsem_d.ap().rearrange("p (c d) -> p c d", c=VINC))
        qfT_l = []
        for e in range(EPC):
            qfT = qpool.tile([128, FDC, NQ], bf, tag=f"qfT{e}")
            nc.sync.dma_start(qfT[:], qfT_d.ap()[e].rearrange("p (c q) -> p c q", c=FDC))
            qfT_l.append(qfT)
        wv = wbig.tile([128, FDC, FD], f8, tag="wbig8")
        nc.sync.dma_start(wv[:], wv_d.ap().rearrange("(c p) d -> p c d", p=128))
        wfc = wbig.tile([128, FDC, FD], f8, tag="wbig8")
        for h in range(2):
            nc.sync.dma_start(
                wfc[:, :, h * 512 : (h + 1) * 512],
                wfc_d.ap()[:, h * 512 : (h + 1) * 512]
                .rearrange("(c p) d -> p c d", p=128))

        # memset partial-chunk tiles read at full 128 partitions
        h1T = apool.tile([128, 3, B20], bf, tag="h1T")
        sT = apool.tile([128, 3, B20], bf, tag="sT")
        avgsT_sb = apool.tile([128, 3, B20], f8, tag="avgsT")
        gsem5T = apool.tile([128, 3, B20], bf, tag="gsem5T")
        nc.vector.memset(h1T[:], 0.0)
        nc.vector.memset(sT[:], 0.0)
        nc.vector.memset(avgsT_sb[:], 0.0)
        nc.vector.memset(gsem5T[:], 0.0)

        _ei = [0]

        def copy_rr(dst, src):
            _ei[0] += 1
            if _ei[0] % 2:
                nc.vector.tensor_copy(dst, src)
            else:
                nc.scalar.copy(dst, src)

        # ---------------- sMLP ----------------
        for mc, (moff, msz) in enumerate(SEMCH):
            ph = pmisc.tile([128, B20], f32, tag="ps_m")
            for kc, (koff, ksz) in enumerate(SEMCH):
                nc.tensor.matmul(ph[0:msz, :], wm1[0:ksz, kc, moff : moff + msz],
                                 ssT[0:ksz, kc, :], start=(kc == 0), stop=(kc == 2))
            lk = spool.tile([128, B20], f32, tag="mlp_lk")
            nc.vector.tensor_scalar(lk[0:msz, :], ph[0:msz, :], bm1c[0:msz, mc : mc + 1],
                                    0.1, op0=ALU.add, op1=ALU.mult)
            nc.vector.tensor_scalar(h1T[0:msz, mc, :], ph[0:msz, :],
                                    bm1c[0:msz, mc : mc + 1], None, op0=ALU.add)
            nc.vector.tensor_tensor(h1T[0:msz, mc, :], h1T[0:msz, mc, :], lk[0:msz, :],
                                    op=ALU.max)
        for mc, (moff, msz) in enumerate(SEMCH):
            ph = pmisc.tile([128, B20], f32, tag="ps_m")
            for kc, (koff, ksz) in enumerate(SEMCH):
                nc.tensor.matmul(ph[0:msz, :], wm2[0:ksz, kc, moff : moff + msz],
                                 h1T[0:ksz, kc, :], start=(kc == 0), stop=(kc == 2))
            nc.vector.tensor_scalar(sT[0:msz, mc, :], ph[0:msz, :],
                                    bm2c[0:msz, mc : mc + 1], None, op0=ALU.add)

        # ---------------- pn_sc = sc/||sc|| ; scT ; pnT sc columns (early) -----
        ssq_sc = spool.tile([B20, 1], f32, tag="sc_ssq")
        sq_sc = spool.tile([B20, FD], bf, tag="sc_sq")
        nc.scalar.activation(sq_sc[:], sc_nat[:], AF.Square, accum_out=ssq_sc[:])
        r_sc = spool.tile([B20, 1], f32, tag="sc_r")
        nc.vector.reciprocal(r_sc[:], ssq_sc[:])
        inv_sc = spool.tile([B20, 1], f32, tag="sc_inv")
        nc.scalar.activation(inv_sc[:], r_sc[:], AF.Sqrt)
        pn_sc = spool.tile([B20, FD], bf, tag="pn_sc")
        nc.vector.tensor_scalar(pn_sc[:], sc_nat[:], inv_sc[:], None, op0=ALU.mult)
        # prime the Exp table (gates + softmax) while DMA streams
        dummy = spool.tile([1, 1], f32, tag="dummy")
        nc.scalar.activation(dummy[:], inv_sc[0:1, 0:1], AF.Exp)
        # sc -> bf16 for the q projection, then transpose on-chip
        sc_bf = spool.tile([B20, FD], bf, tag="sc_bf")
        nc.vector.tensor_copy(sc_bf[:], sc_nat[:])
        scT_t = apool.tile([128, FDC, B20], bf, tag="scT")
        for g in range(2):
            tfull = pbt.tile([128, 1024], bf, tag="tr")
            t = tfull[:, 0 : 4 * B20]
            for i in range(4):
                dc = g * 4 + i
                nc.tensor.transpose(t[:, i * B20 : (i + 1) * B20],
                                    sc_bf[:, dc * 128 : (dc + 1) * 128], ident[0:B20, 0:B20])
            copy_rr(scT_t[:, g * 4 : (g + 1) * 4, :], t[:])
        pnT = apool.tile([128, FDC, EPC * NPROTO], bf, tag="pnT")
        for g in range(2):
            tfull = pbt.tile([128, 1024], bf, tag="tr")
            t = tfull[:, 0 : 4 * B20]
            for i in range(4):
                dc = g * 4 + i
                nc.tensor.transpose(t[:, i * B20 : (i + 1) * B20],
                                    pn_sc[:, dc * 128 : (dc + 1) * 128], ident[0:B20, 0:B20])
            for i in range(4):
                dc = g * 4 + i
                dst = pnT[:, dc, :].rearrange("p (e s) -> p e s", s=NPROTO)[:, :, 0:NW]
                src = t[:, i * B20 : (i + 1) * B20].rearrange("p (e w) -> p e w", w=NW)
                nc.vector.tensor_copy(dst, src)

        # ---------------- banks: avgT-direct + on-chip transposes --------------
        av_bank = pavt.tile([128, FDC + 3, B20], f32, tag="ps_avT")
        avT_ps = av_bank[:, 0:FDC, :]
        asT_ps = av_bank[:, FDC : FDC + 3, :]
        bwT_l, bsmT_l = [], []

        def bank_block(e):
            for dc in range(FDC):
                for c in range(NBC):
                    nc.tensor.matmul(avT_ps[:, dc, e * NW : (e + 1) * NW],
                                     bw_nat[e][:, c, dc * 128 : (dc + 1) * 128],
                                     esel[:, e, e * NW : (e + 1) * NW],
                                     start=(c == 0), stop=(c == NBC - 1))
            bwT = tpool.tile([128, FDC, NB], bf, tag=f"bwT{e}")
            for g in range(4):
                t = pbt.tile([128, 1024], bf, tag="tr")
                for i in range(2):
                    dc = g * 2 + i
                    for c in range(NBC):
                        nc.tensor.transpose(
                            t[:, i * 512 + c * 128 : i * 512 + (c + 1) * 128],
                            bw_nat[e][:, c, dc * 128 : (dc + 1) * 128],
                            ident[:])
                copy_rr(bwT[:, g * 2 : g * 2 + 2, :], t[:])
            bwT_l.append(bwT)
            for sci, (soff, ssz) in enumerate(SEMCH):
                for c in range(NBC):
                    nc.tensor.matmul(asT_ps[0:ssz, sci, e * NW : (e + 1) * NW],
                                     bsm_nat[e][:, c, soff : soff + ssz],
                                     esel[:, e, e * NW : (e + 1) * NW],
                                     start=(c == 0), stop=(c == NBC - 1))
            bsmT = tpool.tile([128, 3, NB], bf, tag=f"bsmT{e}")
            t2p = pbt.tile([128, 1024], bf, tag="tr")
            for sci, (soff, ssz) in enumerate(SEMCH[:2]):
                for c in range(NBC):
                    nc.tensor.transpose(
                        t2p[:, sci * 512 + c * 128 : sci * 512 + (c + 1) * 128],
                        bsm_nat[e][:, c, soff : soff + ssz], ident[:])
            copy_rr(bsmT[:, 0:2, :], t2p[:])
            t3p = pbt.tile([128, 1024], bf, tag="tr")
            soff, ssz = SEMCH[2]
            for c in range(NBC):
                nc.tensor.transpose(t3p[0:ssz, c * 128 : (c + 1) * 128],
                                    bsm_nat[e][:, c, soff : soff + ssz],
                                    ident[:])
            copy_rr(bsmT[0:ssz, 2, :], t3p[0:ssz, 0:512])
            bsmT_l.append(bsmT)

        bank_block(0)

        # qT = (sc@Wq + s@Wqs)^T
        qT_ps = pmisc.tile([128, FDC, B20], f32, tag="ps_m")
        for m in range(FDC):
            for kc in range(FDC):
                nc.tensor.matmul(qT_ps[:, m, :], wq[:, kc, m * 128 : (m + 1) * 128],
                                 scT_t[:, kc, :], start=(kc == 0), stop=False)
            for c in range(3):
                nc.tensor.matmul(qT_ps[:, m, :], wqs[:, c, m * 128 : (m + 1) * 128],
                                 sT[:, c, :], start=False, stop=(c == 2))
        qT = apool.tile([128, FDC, B20], bf, tag="qT")
        nc.vector.tensor_copy(qT[:], qT_ps[:])

        bank_block(1)

        # t1T / t2T
        t1_ps = pmisc.tile([128, FDC, B20], f32, tag="ps_m")
        for m in range(FDC):
            for kc in range(FDC):
                nc.tensor.matmul(t1_ps[:, m, :], wkT[:, kc, m * 128 : (m + 1) * 128],
                                 qT[:, kc, :], start=(kc == 0), stop=(kc == FDC - 1))
        t1c = apool.tile([128, FDC, B20], bf, tag="t1c")
        nc.vector.tensor_copy(t1c[:], t1_ps[:])
        t2_ps = pmisc.tile([128, 3, B20], f32, tag="ps_m")
        for mc, (moff, msz) in enumerate(SEMCH):
            for kc in range(FDC):
                nc.tensor.matmul(t2_ps[0:msz, mc, :], wksT[:, kc, moff : moff + msz],
                                 qT[:, kc, :], start=(kc == 0), stop=(kc == FDC - 1))
        t2c = apool.tile([128, 3, B20], bf, tag="t2c")
        nc.vector.tensor_copy(t2c[:], t2_ps[:])

        bank_block(2)
        bank_block(3)

        # ---------------- avgT copies -> SBUF (cast fp8 for gate matmuls) ------
        avgvT_sb = apool.tile([128, FDC, B20], f8, tag="avgvT")
        nc.vector.tensor_copy(avgvT_sb[:], avT_ps[:])
        nc.vector.tensor_copy(avgsT_sb[:, 0:2, :], asT_ps[:, 0:2, :])
        soff, ssz = SEMCH[2]
        nc.vector.tensor_copy(avgsT_sb[0:ssz, 2, :], asT_ps[0:ssz, 2, :])

        # ---------------- gates directly transposed (fp8) ----------------
        g_bank = pavt.tile([128, FDC + 3, B20], f32, tag="ps_avT")
        gv_ps = g_bank[:, 0:FDC, :]
        gs_ps = g_bank[:, FDC : FDC + 3, :]
        for m in range(FDC):
            for kc in range(FDC):
                nc.tensor.matmul(gv_ps[:, m, :], wvis[:, kc, m * 128 : (m + 1) * 128],
                                 avgvT_sb[:, kc, :], start=(kc == 0), stop=False)
            for c in range(3):
                nc.tensor.matmul(gv_ps[:, m, :], wvis[:, FDC + c, m * 128 : (m + 1) * 128],
                                 avgsT_sb[:, c, :], start=False, stop=False)
            nc.tensor.matmul(gv_ps[:, m, :], bvis_row[0:1, m * 128 : (m + 1) * 128],
                             ones20[:], start=False, stop=True)
        for mc, (moff, msz) in enumerate(SEMCH):
            for kc in range(FDC):
                nc.tensor.matmul(gs_ps[0:msz, mc, :], wsem[:, kc, moff : moff + msz],
                                 avgvT_sb[:, kc, :], start=(kc == 0), stop=False)
            for c in range(3):
                nc.tensor.matmul(gs_ps[0:msz, mc, :], wsem[:, FDC + c, moff : moff + msz],
                                 avgsT_sb[:, c, :], start=False, stop=False)
            nc.tensor.matmul(gs_ps[0:msz, mc, :], bsem_row[0:1, moff : moff + msz],
                             ones20[:], start=False, stop=True)

        # gate = 1 + sigmoid(x) = 1 + 1/(1 + exp(-x)) — Exp shares softmax table
        gvis5T = apool.tile([128, FDC, B20], bf, tag="gvis5T")
        gve = spool.tile([128, FDC, B20], f32, tag="gv_exp")
        nc.scalar.activation(gve[:], gv_ps[:], AF.Exp, scale=-1.0)
        nc.vector.tensor_scalar_add(gve[:], gve[:], 1.0)
        nc.vector.reciprocal(gve[:], gve[:])
        nc.vector.tensor_scalar_add(gvis5T[:], gve[:], 1.0)
        gse = spool.tile([128, 3, B20], f32, tag="gs_exp")
        soff, ssz = SEMCH[2]
        nc.scalar.activation(gse[:, 0:2, :], gs_ps[:, 0:2, :], AF.Exp, scale=-1.0)
        nc.scalar.activation(gse[0:ssz, 2, :], gs_ps[0:ssz, 2, :], AF.Exp, scale=-1.0)
        nc.vector.tensor_scalar_add(gse[:, 0:2, :], gse[:, 0:2, :], 1.0)
        nc.vector.tensor_scalar_add(gse[0:ssz, 2, :], gse[0:ssz, 2, :], 1.0)
        nc.vector.reciprocal(gse[:, 0:2, :], gse[:, 0:2, :])
        nc.vector.reciprocal(gse[0:ssz, 2, :], gse[0:ssz, 2, :])
        nc.vector.tensor_scalar_add(gsem5T[:, 0:2, :], gse[:, 0:2, :], 1.0)
        nc.vector.tensor_scalar_add(gsem5T[0:ssz, 2, :], gse[0:ssz, 2, :], 1.0)

        t1gT = apool.tile([128, FDC, B20], bf, tag="t1gT")
        nc.vector.tensor_tensor(t1gT[:], t1c[:], gvis5T[:], op=ALU.mult)
        t2gT = apool.tile([128, 3, B20], bf, tag="t2gT")
        nc.vector.tensor_tensor(t2gT[:], t2c[:], gsem5T[:], op=ALU.mult)

        # ---------------- attention: stage-parallel ----------------
        def scores_mm(e):
            sc_ps = pscore.tile([NW, NB], f32, tag="ps_sc")
            for dc in range(FDC):
                nc.tensor.matmul(sc_ps[:], t1gT[:, dc, e * NW : (e + 1) * NW],
                                 bwT_l[e][:, dc, :], start=(dc == 0), stop=False)
            for sci, (soff2, ssz2) in enumerate(SEMCH):
                nc.tensor.matmul(sc_ps[:], t2gT[0:ssz2, sci, e * NW : (e + 1) * NW],
                                 bsmT_l[e][0:ssz2, sci, :], start=False, stop=(sci == 2))
            return sc_ps

        def softmax(e, sc_ps):
            # |scores|/32 is bounded well inside f32 exp range — skip max-sub
            attn = ep4.tile([NW, NB], bf, tag="attn")
            sm = ep4.tile([NW, 1], f32, tag="sm")
            nc.scalar.activation(attn[:], sc_ps[:], AF.Exp, scale=1.0 / 32.0,
                                 accum_out=sm[:])
            rs = ep4.tile([NW, 1], f32, tag="rs")
            nc.vector.reciprocal(rs[:], sm[:])
            nc.vector.tensor_scalar(attn[:], attn[:], rs[:], None, op0=ALU.mult)
            return attn

        def attnT_mm(e, attn):
            # 8-wide psum slots keep each transpose output 4-byte aligned
            attnT = ep4.tile([128, NBC, NW], bf, tag="attnT")
            ta_f = pbt.tile([128, 1024], bf, tag="tr")
            for c in range(NBC):
                nc.tensor.transpose(ta_f[:, c * 8 : c * 8 + NW],
                                    attn[:, c * 128 : (c + 1) * 128], ident[0:NW, 0:NW])
            nc.vector.tensor_copy(
                attnT[:],
                ta_f[:, 0 : NBC * 8].rearrange("p (c x) -> p c x", c=NBC)[:, :, 0:NW])
            return attnT

        ugT = apool.tile([128, FDC, B20], f8, tag="ugT")

        def uT_mm(e, attnT):
            uT_ps = pmisc.tile([128, FDC, NW], f32, tag="ps_m")
            for dc in range(FDC):
                for c in range(NBC):
                    nc.tensor.matmul(uT_ps[:, dc, :],
                                     bw_nat[e][:, c, dc * 128 : (dc + 1) * 128],
                                     attnT[:, c, :], start=(c == 0), stop=(c == NBC - 1))
            nc.vector.tensor_tensor(ugT[:, :, e * NW : (e + 1) * NW], uT_ps[:],
                                    gvis5T[:, :, e * NW : (e + 1) * NW], op=ALU.mult)

        sc_ps_l = [None] * EPC
        attn_l = [None] * EPC
        attnT_list = [None] * EPC
        sc_ps_l[0] = scores_mm(0)
        sc_ps_l[1] = scores_mm(1)
        attn_l[0] = softmax(0, sc_ps_l[0])
        attnT_list[0] = attnT_mm(0, attn_l[0])
        sc_ps_l[2] = scores_mm(2)
        attn_l[1] = softmax(1, sc_ps_l[1])
        uT_mm(0, attnT_list[0])
        attnT_list[1] = attnT_mm(1, attn_l[1])
        sc_ps_l[3] = scores_mm(3)
        attn_l[2] = softmax(2, sc_ps_l[2])
        uT_mm(1, attnT_list[1])
        attnT_list[2] = attnT_mm(2, attn_l[2])
        attn_l[3] = softmax(3, sc_ps_l[3])
        uT_mm(2, attnT_list[2])
        attnT_list[3] = attnT_mm(3, attn_l[3])
        uT_mm(3, attnT_list[3])

        # prime the Sqrt table while attention drains
        dummy2 = spool.tile([1, 1], f32, tag="dummy2")
        nc.scalar.activation(dummy2[:], ssq_sc[0:1, 0:1], AF.Sqrt)

        # ---------------- per-query temp/||qf|| scales (from qfT) -------------
        s10_l = []
        for e in range(EPC):
            sq = spool.tile([128, FDC, NQ], bf, tag=f"qsq{e}")
            nc.vector.tensor_tensor(sq[:], qfT_l[e][:], qfT_l[e][:], op=ALU.mult)
            ssq_ps = pmisc.tile([NQ, 1], f32, tag="ps_m")
            for dc in range(FDC):
                nc.tensor.matmul(ssq_ps[:], sq[:, dc, :], ones128[:],
                                 start=(dc == 0), stop=(dc == FDC - 1))
            ssq = ep4.tile([NQ, 1], f32, tag="q_ssq")
            nc.vector.tensor_copy(ssq[:], ssq_ps[:])
            rq = ep4.tile([NQ, 1], f32, tag="q_rq")
            nc.vector.reciprocal(rq[:], ssq[:])
            s10 = qpool.tile([NQ, 1], f32, tag=f"s10_{e}")
            nc.scalar.activation(s10[:], rq[:], AF.Sqrt, scale=float(temp) * float(temp))
            s10_l.append(s10)

        # ---------------- outT = ((u*g) @ Wv)^T (fp8) ----------------
        outT_ps = pmisc.tile([128, FDC, B20], f32, tag="ps_m")
        for m in range(FDC):
            for kc in range(FDC):
                nc.tensor.matmul(outT_ps[:, m, :], wv[:, kc, m * 128 : (m + 1) * 128],
                                 ugT[:, kc, :], start=(kc == 0), stop=(kc == FDC - 1))
        outT = apool.tile([128, FDC, B20], f8, tag="outT")
        nc.vector.tensor_copy(outT[:], outT_ps[:])

        # ---------------- out2 = out@Wfc + sc ; fake (per half) ----------------
        out2 = apool.tile([B20, FD], f32r, tag="out2")
        fk = spool.tile([EPC, FD], f32, tag="fk")
        ssf_h0 = spool.tile([EPC, 1], f32, tag="fk_ssq0")
        ssf_h1 = spool.tile([EPC, 1], f32, tag="fk_ssq1")
        ssf_h = [ssf_h0, ssf_h1]
        sqf = spool.tile([EPC, FD], bf, tag="fk_sq")
        o_bank = pacc.tile([128, 512], f32, tag="ps_acc")
        for h in range(2):
            o2_ps = o_bank[h * 32 : h * 32 + B20, :]
            for kc in range(FDC):
                nc.tensor.matmul(o2_ps, outT[:, kc, :],
                                 wfc[:, kc, h * 512 : (h + 1) * 512],
                                 start=(kc == 0), stop=(kc == FDC - 1))
            nc.vector.tensor_tensor(out2[:, h * 512 : (h + 1) * 512], o2_ps,
                                    sc_nat[:, h * 512 : (h + 1) * 512], op=ALU.add)
            fk_ps_t = pscore.tile([NW, NB], f32, tag="ps_sc")
            fk_ps = fk_ps_t[0:EPC, :]
            nc.tensor.matmul(fk_ps, fifths[:],
                             out2[:, h * 512 : (h + 1) * 512],
                             start=True, stop=True)
            nc.vector.tensor_copy(fk[:, h * 512 : (h + 1) * 512], fk_ps)
            nc.scalar.activation(sqf[:, h * 512 : (h + 1) * 512],
                                 fk[:, h * 512 : (h + 1) * 512], AF.Square,
                                 accum_out=ssf_h[h][:])
        ssf = spool.tile([EPC, 1], f32, tag="fk_ssq")
        nc.vector.tensor_tensor(ssf[:], ssf_h0[:], ssf_h1[:], op=ALU.add)
        rf = spool.tile([EPC, 1], f32, tag="fk_r")
        nc.vector.reciprocal(rf[:], ssf[:])
        inv_f = spool.tile([EPC, 1], f32, tag="fk_inv")
        nc.scalar.activation(inv_f[:], rf[:], AF.Sqrt)
        pn_fk = spool.tile([EPC, FD], bf, tag="pn_fk")
        nc.vector.tensor_scalar(pn_fk[:], fk[:], inv_f[:], None, op0=ALU.mult)
        tf_f = pbt.tile([128, 1024], bf, tag="tr")
        tf = tf_f[:, 0 : FDC * EPC]
        for dc in range(FDC):
            nc.tensor.transpose(tf[:, dc * EPC : (dc + 1) * EPC],
                                pn_fk[:, dc * 128 : (dc + 1) * 128], ident[0:EPC, 0:EPC])
        dst = pnT[:].rearrange("p c (e s) -> p c e s", s=NPROTO)[:, :, :, NW]
        nc.vector.tensor_copy(dst, tf[:].rearrange("p (c e) -> p c e", e=EPC))

        # ---------------- logits + per-query scale ----------------
        lg_ps = pmisc.tile([NQ, EPC * NPROTO], f32, tag="ps_m")
        for e in range(EPC):
            for dc in range(FDC):
                nc.tensor.matmul(lg_ps[:, e * NPROTO : (e + 1) * NPROTO],
                                 qfT_l[e][:, dc, :],
                                 pnT[:, dc, e * NPROTO : (e + 1) * NPROTO],
                                 start=(dc == 0), stop=(dc == FDC - 1))
        lg = spool.tile([NQ, EPC * NPROTO], f32, tag="lg")
        for e in range(EPC):
            nc.vector.tensor_scalar(lg[:, e * NPROTO : (e + 1) * NPROTO],
                                    lg_ps[:, e * NPROTO : (e + 1) * NPROTO],
                                    s10_l[e][:], None, op0=ALU.mult)
        nc.sync.dma_start(out_d.ap().rearrange("e q s -> q e s"),
                          lg[:].rearrange("q (e s) -> q e s", s=NPROTO))

    nc.finalize()
    return nc


def _pack_chunks(a, p=128):
    """[R, C] -> [p, (R//p)*C] with chunk-major packing (R = n*p)."""
    n = a.shape[0] // p
    return np.ascontiguousarray(
        a.reshape(n, p, a.shape[1]).transpose(1, 0, 2).reshape(p, n * a.shape[1]))


def _prep_shared(inputs):
    """dtype conversion, transposes, padding, aux blob packing (host side)."""
    def b(a):
        return np.ascontiguousarray(np.asarray(a).astype(BF16))

    def b8(a):
        return np.ascontiguousarray(np.asarray(a).astype(FP8))

    def padr(a, n):
        p = np.zeros((n - a.shape[0],) + a.shape[1:], a.dtype)
        return np.ascontiguousarray(np.concatenate([a, p], axis=0))

    f32 = np.float32
    aux_bf = np.zeros((128, 2069), f32)
    aux_bf[:, 0:128] = np.eye(128, dtype=f32)
    for e in range(EPC):
        aux_bf[:, 188 + e * B20 + e * NW : 188 + e * B20 + (e + 1) * NW] = 1.0 / NB
    wm1 = padr(np.asarray(inputs["Wm1"], f32), SEMP).reshape(3, 128, SEM)
    wm2 = padr(np.asarray(inputs["Wm2"], f32), SEMP).reshape(3, 128, SEM)
    for c in range(3):
        aux_bf[:, 268 + c * SEM : 268 + (c + 1) * SEM] = wm1[c]
        aux_bf[:, 1168 + c * SEM : 1168 + (c + 1) * SEM] = wm2[c]
    aux_bf[:, 2068] = 1.0

    aux_f8 = np.zeros((128, 1504), f32)
    aux_f8[0, 160:180] = 1.0
    aux_f8[0, 180:1204] = np.asarray(inputs["bvis"], f32).reshape(-1)
    aux_f8[0, 1204:1504] = np.asarray(inputs["bsem"], f32).reshape(-1)

    aux_f32 = np.zeros((128, 1035), f32)
    bm1 = np.asarray(inputs["bm1"], f32).reshape(-1)
    bm2 = np.asarray(inputs["bm2"], f32).reshape(-1)
    for c, (off, sz) in enumerate(SEMCH):
        aux_f32[0:sz, 1024 + c] = bm1[off : off + sz]
        aux_f32[0:sz, 1027 + c] = bm2[off : off + sz]

    fifths = np.zeros((B20, EPC), f32)
    for e in range(EPC):
        fifths[e * NW : (e + 1) * NW, e] = 1.0 / NW

    shared = {
        "fifths": fifths,
        "Wq": b(inputs["Wq"]),
        "Wqs_pad": b(padr(np.asarray(inputs["Wqs"], f32), SEMP)),
        "WkT": b(np.asarray(inputs["Wk"], f32).T),
        "WksT_pack": b(_pack_chunks(np.asarray(inputs["Wks"], f32).T)),
        "Wv": b8(inputs["Wv"]),
        "Wfc": b8(inputs["Wfc"]),
        "Wvis_pad": b8(padr(np.asarray(inputs["Wvis"], f32), VINP)),
        "Wsem_pack": b8(_pack_chunks(padr(np.asarray(inputs["Wsem"], f32), VINP))),
    }
    return shared, aux_bf, aux_f8, aux_f32


def kernel(**inputs):
    from concourse.bass_utils import run_bass_kernel_spmd

    temp = float(np.asarray(inputs["temp"]))
    key = ("v12", temp)
    if key not in _MODULE_CACHE:
        _MODULE_CACHE[key] = _build_module(temp)
    nc = _MODULE_CACHE[key]

    shared, aux_bf, aux_f8, aux_f32 = _prep_shared(inputs)
    sc_f = np.asarray(inputs["support_center"], np.float32)
    ss_f = np.asarray(inputs["support_seman"], np.float32)
    bw_f = np.asarray(inputs["base_weights"], np.float32)
    bsm_f = np.asarray(inputs["base_seman"], np.float32)
    qf_f = np.asarray(inputs["query_feature"], np.float32)

    in_maps = []
    for cid in range(NCORES):
        lo, hi = cid * EPC, (cid + 1) * EPC
        sc20 = np.ascontiguousarray(sc_f[lo:hi].reshape(B20, FD))
        ss20 = ss_f[lo:hi].reshape(B20, SEM)
        abf = aux_bf.copy()
        for c, (off, sz) in enumerate(SEMCH):
            abf[0:sz, 128 + c * B20 : 128 + (c + 1) * B20] = ss20[:, off : off + sz].T
        af32 = aux_f32.copy()
        af32[0:B20, 0:FD] = sc20
        m = dict(shared)
        m["aux_bf"] = np.ascontiguousarray(abf.astype(BF16))
        m["aux_f8"] = np.ascontiguousarray(aux_f8.astype(FP8))
        m["aux_f32"] = np.ascontiguousarray(af32)
        m["bw"] = np.ascontiguousarray(bw_f[lo:hi].astype(BF16))
        m["bsm_pack"] = np.ascontiguousarray(np.stack(
            [_pack_chunks(bsm_f[lo + e]) for e in range(EPC)]).astype(BF16))
        m["qfT_pack"] = np.ascontiguousarray(np.stack(
            [_pack_chunks(qf_f[lo + e].T) for e in range(EPC)]).astype(BF16))
        in_maps.append(m)

    res = run_bass_kernel_spmd(nc, in_maps, core_ids=list(range(NCORES)))
    out = np.concatenate([res.results[c]["out"] for c in range(NCORES)], axis=0)
    return out.astype(np.float32)
